# revision 4
# baseline (speedup 1.0000x reference)
"""Trainium2 Bass kernel for BSplineLayer: y = BSpline(knots, coeffs, k=3)((x - min(x)) / (max(x) - min(x) + 1e-8)).

The reference clips the de Boor interval to [3, 3], so the layer is one cubic
P(xn) evaluated everywhere; normalization folds into composed raw-x
coefficients q_i, evaluated as y = (x^2 + alpha)*(q3*x + q2) + delta.

This version cuts DVE work with fp16 fast paths (measured DVE modes: ~0.5
cyc/elem for fp32 tensor_scalar [2x_2P], ~0.5 for fp16 tensor_tensor [2x_1P],
~0.25 for all-fp16 tensor_scalar [4x]; tensor_reduce and STT are always ~1
cyc/elem):

- Phase 1: x tiles stream through a 2-deep fp32 staging pool; DVE casts each
  to a kept fp16 copy xh (2x), folds running max/min accumulators with fp16
  TTs (2x), and finishes with two small reduces -> (max, -min) pair.  ACT
  squares each staged tile into a kept fp16 xsq.  Local stats complete around
  t=70us, just before the ncfw stream slot opens (warm AllReduce enqueued at
  t~8 absorbs the ~39us first-collective barrier; the real 8-byte
  AllReduce(max) lands ~95us).
- Phase 2: per [128,2048] chunk: t1 = fp16(q3*xh + q2) (DVE TS @4x),
  u = (xsq + alpha)*t1 (DVE STT in place over xsq), y = u + delta (ACT
  Identity, fp16->fp32), DMA out.  DVE post-collective work is ~45us vs 53us
  for the fp32 baseline; output DMA (~47us) is the post-collective floor.

fp16 rounding of xh/xsq/t1/u costs ~1.7e-3 relative error (vs 2e-2 budget).
"""

import sys

sys.path.insert(0, "/opt/trn_rl_repo")

import numpy as np

N_CORES = 8
ROWS, COLS = 8192, 4096
R_CORE = ROWS // N_CORES          # 1024 rows per core
P = 128                           # SBUF partitions
N_TILES = R_CORE // P             # 8 tiles of [128, 4096] per core
CHUNK = 2048                      # phase-2 free-dim chunk
FREE = N_TILES * COLS             # 32768 free elems per partition
DEGREE = 3

_CACHE = {}


def _expand_cubic(knots: np.ndarray, coeffs: np.ndarray) -> np.ndarray:
    """Expand de Boor at interval m=3 into monomial coeffs [a0, a1, a2, a3] (float64)."""
    t = np.asarray(knots, dtype=np.float64)
    c = np.asarray(coeffs, dtype=np.float64)
    k = DEGREE
    m = k  # reference clips searchsorted result to [k, n-1] with n-1 == k
    pm = np.polynomial.polynomial
    d = [np.array([c[m - k + j]], dtype=np.float64) for j in range(k + 1)]
    for r in range(1, k + 1):
        for j in range(k, r - 1, -1):
            tl = t[m - k + j]
            tr = t[m + j + 1 - r]
            inv = 1.0 / (tr - tl)
            alpha = np.array([-tl * inv, inv])
            one_m = np.array([1.0 + tl * inv, -inv])
            d[j] = pm.polyadd(pm.polymul(one_m, d[j - 1]), pm.polymul(alpha, d[j]))
    a = np.zeros(4, dtype=np.float64)
    a[: len(d[k])] = d[k]
    return a


def _build_program():
    import concourse.bass as bass
    import concourse.tile as tile
    from concourse import bacc, bass_isa, mybir

    f32 = mybir.dt.float32
    f16 = mybir.dt.float16
    OP = mybir.AluOpType
    AX = mybir.AxisListType
    AF = mybir.ActivationFunctionType

    nc = bacc.Bacc("TRN2", target_bir_lowering=False, debug=False, num_devices=N_CORES)
    x_ext = nc.declare_dram_parameter("x", [R_CORE, COLS], f32, isOutput=False)
    ac_ext = nc.declare_dram_parameter("ac", [1, 4], f32, isOutput=False)
    y_ext = nc.declare_dram_parameter("y", [R_CORE, COLS], f32, isOutput=True)

    with tile.TileContext(nc) as tc:
        with (
            tc.tile_pool(name="stage", bufs=2) as stage,
            tc.tile_pool(name="keep", bufs=1) as keep,
            tc.tile_pool(name="acc", bufs=1) as accp,
            tc.tile_pool(name="t1p", bufs=3) as t1p,
            tc.tile_pool(name="yp", bufs=2) as yp,
            tc.tile_pool(name="small", bufs=1) as small,
            tc.tile_pool(name="dram", bufs=1, space="DRAM") as dram,
        ):
            # Warm the collective path (ncfw queue/ring setup + core-skew
            # sync) concurrently with phase 1 so the real AllReduce is cheap.
            warm_in = dram.tile([1, 2], f32)
            warm_out = dram.tile([1, 2], f32)
            nc.gpsimd.collective_compute(
                "AllReduce", OP.max,
                replica_groups=[list(range(N_CORES))],
                ins=[warm_in[:].opt()], outs=[warm_out[:].opt()],
            )

            # kept fp16 copies of the whole shard
            xh = keep.tile([P, FREE], f16, tag="xh")
            xsq = keep.tile([P, FREE], f16, tag="xsq")
            # running max/min accumulators (fp16, half-tile wide)
            mx = accp.tile([P, CHUNK], f16, tag="mx")
            mn = accp.tile([P, CHUNK], f16, tag="mn")

            # ---------------- phase 1: load, cast, square, fold ----------------
            for t in range(N_TILES):
                xt = stage.tile([P, COLS], f32, tag="xs")
                nq = 4 if t == 0 else (2 if t == 1 else 1)
                QT = COLS // nq
                for q in range(nq):
                    sl = slice(q * QT, (q + 1) * QT)
                    nc.sync.dma_start(out=xt[:, sl],
                                      in_=x_ext[t * P:(t + 1) * P, sl])
                    nc.vector.tensor_scalar(
                        xh[:, t * COLS + q * QT:t * COLS + (q + 1) * QT],
                        xt[:, sl], 1.0, None, op0=OP.mult)
                    nc.scalar.activation(
                        xsq[:, t * COLS + q * QT:t * COLS + (q + 1) * QT],
                        xt[:, sl], AF.Square, bias=0.0, scale=1.0)
                h0 = xh[:, t * COLS:t * COLS + CHUNK]
                h1 = xh[:, t * COLS + CHUNK:(t + 1) * COLS]
                if t == 0:
                    nc.vector.tensor_tensor(mx[:], h0, h1, op=OP.max)
                    nc.vector.tensor_tensor(mn[:], h0, h1, op=OP.min)
                else:
                    nc.vector.tensor_tensor(mx[:], mx[:], h0, op=OP.max)
                    nc.vector.tensor_tensor(mx[:], mx[:], h1, op=OP.max)
                    nc.vector.tensor_tensor(mn[:], mn[:], h0, op=OP.min)
                    nc.vector.tensor_tensor(mn[:], mn[:], h1, op=OP.min)

            pk = small.tile([P, 2], f32)
            nc.vector.tensor_reduce(pk[:, 0:1], mx[:], axis=AX.X, op=OP.max)
            rmn = small.tile([P, 1], f32)
            nc.vector.tensor_reduce(rmn[:], mn[:], axis=AX.X, op=OP.min)
            nc.vector.tensor_scalar_mul(pk[:, 1:2], rmn[:], -1.0)

            # cross-partition: every partition gets (local_max, -local_min)
            par = small.tile([P, 2], f32)
            nc.gpsimd.partition_all_reduce(par[:], pk[:], channels=P,
                                           reduce_op=bass_isa.ReduceOp.max)

            # cross-core: AllReduce(max) of the pair
            cc_in = dram.tile([1, 2], f32)
            cc_out = dram.tile([1, 2], f32)
            nc.sync.dma_start(out=cc_in[:], in_=par[0:1, 0:2])
            nc.gpsimd.collective_compute(
                "AllReduce", OP.max,
                replica_groups=[list(range(N_CORES))],
                ins=[cc_in[:].opt()], outs=[cc_out[:].opt()],
            )
            GG = small.tile([P, 2], f32)
            nc.sync.dma_start(out=GG[:], in_=cc_out[:].partition_broadcast(P))

            # host constants in: ac = [e2a=a2/a3, e1a=a1/a3, a3, a0]
            ac_sb = small.tile([1, 4], f32)
            nc.sync.dma_start(out=ac_sb[:], in_=ac_ext[:])
            AC = small.tile([P, 4], f32)
            nc.gpsimd.partition_broadcast(AC[:], ac_sb[:])
            e2a, e1a, a3c, a0c = (AC[:, i:i + 1] for i in range(4))

            # ------- device scalars: normalization + composed coefficients -------
            # s = 1/(gmax + gnm + eps); b = gnm*s    (gnm = -gmin)
            # y = (xsq + d1)*(q3*x + q2) + delta
            cf = small.tile([P, 8], f32)
            d2c, d1c, q3c, q0c, g1c, g2c, alc = (cf[:, i:i + 1] for i in range(7))
            tmp = small.tile([P, 10], f32)
            dd, s_, b_, u, v, w, s2, p_, de_, _sp = (tmp[:, i:i + 1] for i in range(10))

            nc.vector.scalar_tensor_tensor(dd, GG[:, 0:1], 1e-8, GG[:, 1:2],
                                           op0=OP.add, op1=OP.add)      # d = range+eps
            nc.vector.reciprocal(s_, dd)
            nc.vector.tensor_tensor(b_, GG[:, 1:2], s_, op=OP.mult)     # b = gnm*s

            nc.vector.tensor_scalar_mul(u, b_, 3.0)                     # u = 3b
            nc.vector.tensor_tensor(v, u, e2a, op=OP.add)               # v = 3b+e2a
            nc.vector.tensor_tensor(d2c, v, dd, op=OP.mult)             # d2

            nc.vector.scalar_tensor_tensor(w, e2a, 2.0, u, op0=OP.mult, op1=OP.add)  # w = 2e2a+3b
            nc.vector.tensor_tensor(w, w, b_, op=OP.mult)
            nc.vector.tensor_tensor(w, w, e1a, op=OP.add)               # (3b+2e2a)b+e1a
            nc.vector.tensor_tensor(v, dd, dd, op=OP.mult)              # v = d^2
            nc.vector.tensor_tensor(d1c, w, v, op=OP.mult)              # d1 = alpha

            nc.vector.tensor_tensor(s2, s_, s_, op=OP.mult)
            nc.vector.tensor_tensor(u, s2, s_, op=OP.mult)              # s^3
            nc.vector.tensor_tensor(q3c, u, a3c, op=OP.mult)            # q3

            # q2 first: it (with q3/d1 above) unblocks the phase-2 DVE ops
            nc.vector.tensor_tensor(g1c, d2c, q3c, op=OP.mult)          # q2

            nc.vector.tensor_tensor(p_, b_, e2a, op=OP.add)             # b+e2a
            nc.vector.tensor_tensor(g2c, p_, a3c, op=OP.mult)
            nc.vector.tensor_tensor(p_, p_, b_, op=OP.mult)
            nc.vector.tensor_tensor(p_, p_, e1a, op=OP.add)
            nc.vector.tensor_tensor(p_, p_, b_, op=OP.mult)
            nc.vector.tensor_tensor(p_, p_, a3c, op=OP.mult)
            nc.vector.tensor_tensor(q0c, p_, a0c, op=OP.add)            # q0
            nc.vector.tensor_tensor(de_, g1c, d1c, op=OP.mult)
            nc.vector.tensor_tensor(de_, q0c, de_, op=OP.subtract)      # delta

            # ACT-owned copy of delta (wait-slot limit workaround)
            actsb = small.tile([P, 1], f32)
            nc.scalar.copy(actsb[:, 0:1], de_)

            # ---------------- phase 2: evaluate + store ----------------
            total_chunks = FREE // CHUNK
            for ci in range(total_chunks):
                t, h = divmod(ci, COLS // CHUNK)
                sl = slice(ci * CHUNK, (ci + 1) * CHUNK)
                t1h = t1p.tile([P, CHUNK], f16, tag="t1")
                nc.vector.tensor_scalar(t1h[:], xh[:, sl], q3c, g1c,
                                        op0=OP.mult, op1=OP.add)
                nc.vector.scalar_tensor_tensor(xsq[:, sl], xsq[:, sl], d1c, t1h[:],
                                               op0=OP.add, op1=OP.mult)
                yc = yp.tile([P, CHUNK], f32, tag="y")
                nc.scalar.activation(yc[:], xsq[:, sl], AF.Identity,
                                     bias=actsb[:, 0:1], scale=1.0)
                nc.sync.dma_start(
                    out=y_ext[t * P:(t + 1) * P, h * CHUNK:(h + 1) * CHUNK],
                    in_=yc[:])

    nc.compile()
    return nc


def kernel(x: np.ndarray, knots: np.ndarray, coeffs: np.ndarray) -> np.ndarray:
    from concourse.bass_utils import run_bass_kernel_spmd

    x = np.ascontiguousarray(np.asarray(x, dtype=np.float32))
    assert x.shape == (ROWS, COLS), x.shape

    a = _expand_cubic(knots, coeffs)
    a3 = a[3] if abs(a[3]) > 1e-30 else 1e-30
    ac = np.array([[a[2] / a3, a[1] / a3, a3, a[0]]], dtype=np.float32)

    if "nc" not in _CACHE:
        _CACHE["nc"] = _build_program()
    nc = _CACHE["nc"]

    shards = [x[i * R_CORE:(i + 1) * R_CORE] for i in range(N_CORES)]
    in_maps = [{"x": s, "ac": ac} for s in shards]

    import os
    trace = bool(int(os.environ.get("KERNEL_TRACE", "0")))
    res = run_bass_kernel_spmd(nc, in_maps, core_ids=list(range(N_CORES)),
                               trace=trace)
    if trace and res.exec_time_ns is not None:
        print(f"HW exec time: {res.exec_time_ns} ns")
        _CACHE["last_exec_time_ns"] = res.exec_time_ns
        _CACHE["last_trace"] = res.instructions_and_trace

    out = np.empty((ROWS, COLS), dtype=np.float32)
    for i in range(N_CORES):
        out[i * R_CORE:(i + 1) * R_CORE] = res.results[i]["y"]
    return out
